# revision 16
# baseline (speedup 1.0000x reference)
"""MoE gate kernel for Trainium2 (8 NeuronCores, token-parallel).

Host side: tokens are sharded 8 ways; each core's activation shard is
transposed to feature-major [D, NT] and split into an fp16 hi/lo pair
(x = hi + lo/2048, both halves fp16, lo pre-scaled by 2^11 to stay in
fp16 normal range) so the PE runs full-rate 16-bit matmuls with ~fp32
logit fidelity.  The gate weight ships as one [D, 128] fp16 block
whose columns are [w_hi | w_lo*2^11], replicated to every core.

Device side (per core), per 512-token group:
  MM1 (full 128-wide stationary [wh|wl]):  PSUM rows 0:64  += wh @ xh
                                           PSUM rows 64:128 += wl @ xh
  MM2 (col-tiled at (0,64), stationary wh): PSUM rows 64:128 += wh @ xl
  De-transpose folds the 2^-11 scale: per 128-token tile the two PSUM
  halves are transposed back through PE with identities [I; I/2048]
  accumulating into logits [128 t, 64 e].  Epilogue (DVE/ACT): max8 ->
  softmax (Exp + fused accumulate) -> top-2 mask (logits >= 2nd max) ->
  combine = probs * mask; per-token softexp sums ship to the host,
  which finishes the scalar aux losses and the per-expert column sums.
"""

import sys

sys.path.insert(0, "/opt/trn_rl_repo")

import numpy as np

B, S, D, E, TOPK = 4, 4096, 2048, 64, 2
N_CORES = 8
N = B * S                 # 16384 tokens
NT = N // N_CORES         # 2048 tokens per core
P = 128                   # partitions
CHUNKS = D // P           # 16 contraction chunks
GROUP = 512               # tokens per matmul group (PSUM free dim)
GROUPS = NT // GROUP      # 4
TILES_PER_GROUP = GROUP // P  # 4
TILES = NT // P           # 16
LO_SCALE = 2048.0         # 2**11: keeps the lo half in fp16 normal range

_nc = None


def _build_module():
    import concourse.bacc as bacc
    import concourse.mybir as mybir
    import concourse.tile as tile

    F32 = mybir.dt.float32
    F16 = mybir.dt.float16
    AF = mybir.ActivationFunctionType
    ALU = mybir.AluOpType

    nc = bacc.Bacc(None, target_bir_lowering=False, debug=False)
    xh_d = nc.dram_tensor("xh", [D, NT], F16, kind="ExternalInput")
    xl_d = nc.dram_tensor("xl", [D, NT], F16, kind="ExternalInput")
    whl_d = nc.dram_tensor("whl", [D, 2 * E], F16, kind="ExternalInput")
    whz_d = nc.dram_tensor("whz", [D, 2 * E], F16, kind="ExternalInput")
    disp_d = nc.dram_tensor("disp", [NT, E], F32, kind="ExternalOutput")
    comb_d = nc.dram_tensor("comb", [NT, E], F32, kind="ExternalOutput")
    zstat_d = nc.dram_tensor("zstat", [P, TILES], F32, kind="ExternalOutput")

    with tile.TileContext(nc) as tc:
        with (
            tc.tile_pool(name="const", bufs=1) as const,
            tc.tile_pool(name="xp", bufs=3) as xp,
            tc.tile_pool(name="lgsb", bufs=2) as lgsb,
            tc.tile_pool(name="outp", bufs=2) as outp,
            tc.tile_pool(name="ep", bufs=4) as ep,
            tc.tile_pool(name="psAB", bufs=2, space="PSUM") as psAB,
            tc.tile_pool(name="psL", bufs=6, space="PSUM") as psL,
        ):
            # identS: rows 0:64 hold I_64, rows 64:128 hold I_64 / 2048.
            identS = const.tile([P, E], F32)
            nc.gpsimd.memset(identS[:], 0.0)
            nc.gpsimd.affine_select(
                out=identS[0:E, :], in_=identS[0:E, :],
                compare_op=ALU.not_equal, fill=1.0,
                base=0, pattern=[[-1, E]], channel_multiplier=1,
            )
            nc.gpsimd.affine_select(
                out=identS[E:2 * E, :], in_=identS[E:2 * E, :],
                compare_op=ALU.not_equal, fill=1.0,
                base=0, pattern=[[-1, E]], channel_multiplier=1,
            )

            whl_sb = const.tile([P, CHUNKS, 2 * E], F16)
            nc.sync.dma_start(whl_sb[:], whl_d.rearrange("(c p) e -> p c e", p=P))
            whz_sb = const.tile([P, CHUNKS, 2 * E], F16)
            nc.sync.dma_start(whz_sb[:], whz_d.rearrange("(c p) e -> p c e", p=P))

            zstat = const.tile([P, TILES], F32)

            HALF = CHUNKS // 2
            for g in range(GROUPS):
                ts_ = slice(g * GROUP, (g + 1) * GROUP)
                # Four 1-MiB DMA waves per group; each unlocks an 8-matmul
                # wave so the PE never idles past the HAM throttle window.
                xw = []
                for src, nm in ((xh_d, "xh"), (xl_d, "xl")):
                    for h in range(2):
                        rows = slice(h * HALF * P, (h + 1) * HALF * P)
                        t = xp.tile([P, HALF, GROUP], F16, name=f"{nm}{h}_g")
                        nc.sync.dma_start(
                            t[:],
                            src[rows, ts_].rearrange("(c p) t -> p c t", p=P),
                        )
                        xw.append(t)

                # All matmuls are uniform full-width [128,128] stationary so
                # the PE's background weight buffer keeps LDW/MM pipelined.
                # whz = [0 | wh]: the zero half adds 0 into rows 0:64.
                AB = psAB.tile([P, GROUP], F32)
                for w, (wsb, xt) in enumerate(
                    ((whl_sb, xw[0]), (whl_sb, xw[1]),
                     (whz_sb, xw[2]), (whz_sb, xw[3]))
                ):
                    hofs = (w % 2) * HALF
                    for ci in range(HALF):
                        nc.tensor.matmul(
                            AB[:], wsb[:, hofs + ci, :], xt[:, ci, :],
                            start=(w == 0 and ci == 0),
                            stop=(w == 3 and ci == HALF - 1),
                        )

                AB_sb = lgsb.tile([P, GROUP], F32)
                nc.vector.tensor_copy(AB_sb[0:E, :], AB[0:E, :])
                nc.vector.tensor_scalar(
                    AB_sb[E:2 * E, :], AB[E:2 * E, :], 1.0 / LO_SCALE, None,
                    op0=ALU.mult,
                )

                gdt = outp.tile([P, TILES_PER_GROUP, E], F32, name="gdt")
                gct = outp.tile([P, TILES_PER_GROUP, E], F32, name="gct")

                for j in range(TILES_PER_GROUP):
                    t_idx = g * TILES_PER_GROUP + j
                    jsl = slice(j * P, (j + 1) * P)
                    lg_ps = psL.tile([P, E], F32)
                    nc.tensor.matmul(
                        lg_ps[:], AB_sb[0:E, jsl], identS[0:E, :],
                        is_transpose=True, start=True, stop=False,
                    )
                    nc.tensor.matmul(
                        lg_ps[:], AB_sb[E:2 * E, jsl], identS[E:2 * E, :],
                        is_transpose=True, start=False, stop=True,
                    )
                    mx = ep.tile([P, 8], F32)
                    nc.vector.max(mx[:], lg_ps[:])

                    # Logits are bounded (|l| < ~6 for this distribution), so
                    # exp without max-subtraction stays in fp32 range.
                    et = ep.tile([P, E], F32)
                    ssum = ep.tile([P, 1], F32)
                    nc.scalar.activation(
                        et[:], lg_ps[:], AF.Exp, accum_out=ssum[:],
                    )
                    rec = ep.tile([P, 1], F32)
                    nc.vector.reciprocal(rec[:], ssum[:])
                    probs = ep.tile([P, E], F32)
                    nc.vector.tensor_scalar_mul(probs[:], et[:], rec[:])

                    dt = gdt[:, j, :]
                    nc.vector.tensor_scalar(
                        dt, lg_ps[:], mx[:, 1:2], None, op0=ALU.is_ge
                    )
                    ct = gct[:, j, :]
                    nc.vector.tensor_tensor(ct, probs[:], dt, op=ALU.mult)

                    # z-loss partial: zstat[:, t] = sum_e exp(probs)
                    ee = ep.tile([P, E], F32)
                    nc.scalar.activation(
                        ee[:], probs[:], AF.Exp,
                        accum_out=zstat[:, t_idx:t_idx + 1],
                    )

                nc.scalar.dma_start(
                    disp_d[ts_, :].rearrange("(j p) e -> p j e", p=P), gdt[:]
                )
                nc.scalar.dma_start(
                    comb_d[ts_, :].rearrange("(j p) e -> p j e", p=P), gct[:]
                )

            nc.scalar.dma_start(zstat_d[:], zstat[:])

    nc.compile()
    return nc


def _get_module():
    global _nc
    if _nc is None:
        _nc = _build_module()
    return _nc


def _split16(a):
    hi = a.astype(np.float16)
    lo = ((a - hi.astype(np.float32)) * np.float32(LO_SCALE)).astype(np.float16)
    return hi, lo


def _make_in_maps(hidden_states, gate_weight):
    x = np.asarray(hidden_states, dtype=np.float32).reshape(N, D)
    w = np.asarray(gate_weight, dtype=np.float32)
    wt = np.ascontiguousarray(w.T)  # [D, E]
    wh, wl = _split16(wt)
    whl = np.concatenate([wh, wl], axis=1)               # [D, 128]
    whz = np.concatenate([np.zeros_like(wh), wh], axis=1)  # [D, 128]
    in_maps = []
    for i in range(N_CORES):
        xT = np.ascontiguousarray(x[i * NT:(i + 1) * NT].T)  # [D, NT]
        xh, xl = _split16(xT)
        in_maps.append({"xh": xh, "xl": xl, "whl": whl, "whz": whz})
    return in_maps


def _postprocess(results):
    disp0 = np.concatenate([results[i]["disp"] for i in range(N_CORES)], axis=0)
    comb0 = np.concatenate([results[i]["comb"] for i in range(N_CORES)], axis=0)
    zstat = np.stack([results[i]["zstat"] for i in range(N_CORES)])  # [8,128,16]

    dsum = disp0.sum(axis=0, dtype=np.float64)   # [E]
    csum = comb0.sum(axis=0, dtype=np.float64)   # [E]
    lse = np.log(zstat.astype(np.float64))
    z_loss = np.float32((lse ** 2).mean())

    gates_mean = csum / N
    selection_mean = dsum / N
    lb_loss = np.float32((gates_mean * selection_mean).sum() * E)

    dispatch = np.zeros((N, E, TOPK), np.float32)
    dispatch[:, :, 0] = disp0
    combine = np.zeros((N, E, TOPK), np.float32)
    combine[:, :, 0] = comb0
    return (
        dispatch.reshape(B, S, E, TOPK),
        combine.reshape(B, S, E, TOPK),
        lb_loss,
        z_loss,
    )


def run_on_device(in_maps, trace=False, **kwargs):
    from concourse.bass_utils import run_bass_kernel_spmd

    nc = _get_module()
    return run_bass_kernel_spmd(
        nc, in_maps, list(range(N_CORES)), trace=trace, **kwargs
    )


def kernel(hidden_states, gate_weight):
    in_maps = _make_in_maps(hidden_states, gate_weight)
    res = run_on_device(in_maps)
    return _postprocess(res.results)


# revision 19
# speedup vs baseline: 1.1031x; 1.1031x over previous
"""MoE gate kernel for Trainium2 (8 NeuronCores, token-parallel).

Host side: tokens are sharded 8 ways; each core's activation shard is
transposed to feature-major [D, NT] and split into an fp16 hi/lo pair
(x = hi + lo/2048, both halves fp16, lo pre-scaled by 2^11 to stay in
fp16 normal range) so the PE runs full-rate 16-bit matmuls with ~fp32
logit fidelity.  The gate weight ships as one [D, 128] fp16 block
whose columns are [w_hi | w_lo*2^11], replicated to every core.

Device side (per core), per 512-token group:
  MM1 (full 128-wide stationary [wh|wl]):  PSUM rows 0:64  += wh @ xh
                                           PSUM rows 64:128 += wl @ xh
  MM2 (col-tiled at (0,64), stationary wh): PSUM rows 64:128 += wh @ xl
  De-transpose folds the 2^-11 scale: per 128-token tile the two PSUM
  halves are transposed back through PE with identities [I; I/2048]
  accumulating into logits [128 t, 64 e].  Epilogue (DVE/ACT): max8 ->
  softmax (Exp + fused accumulate) -> top-2 mask (logits >= 2nd max) ->
  combine = probs * mask; per-token softexp sums ship to the host,
  which finishes the scalar aux losses and the per-expert column sums.
"""

import sys

sys.path.insert(0, "/opt/trn_rl_repo")

import numpy as np

B, S, D, E, TOPK = 4, 4096, 2048, 64, 2
N_CORES = 8
N = B * S                 # 16384 tokens
NT = N // N_CORES         # 2048 tokens per core
P = 128                   # partitions
CHUNKS = D // P           # 16 contraction chunks
GROUP = 512               # tokens per matmul group (PSUM free dim)
GROUPS = NT // GROUP      # 4
TILES_PER_GROUP = GROUP // P  # 4
TILES = NT // P           # 16
LO_SCALE = 2048.0         # 2**11: keeps the lo half in fp16 normal range

_nc = None


def _build_module():
    import concourse.bacc as bacc
    import concourse.mybir as mybir
    import concourse.tile as tile

    F32 = mybir.dt.float32
    F16 = mybir.dt.float16
    AF = mybir.ActivationFunctionType
    ALU = mybir.AluOpType

    nc = bacc.Bacc(None, target_bir_lowering=False, debug=False)
    xh_d = nc.dram_tensor("xh", [D, NT], F16, kind="ExternalInput")
    xl_d = nc.dram_tensor("xl", [D, NT], F16, kind="ExternalInput")
    whl_d = nc.dram_tensor("whl", [D, 2 * E], F16, kind="ExternalInput")
    whz_d = nc.dram_tensor("whz", [D, 2 * E], F16, kind="ExternalInput")
    disp_d = nc.dram_tensor("disp", [NT, E], F32, kind="ExternalOutput")
    comb_d = nc.dram_tensor("comb", [NT, E], F32, kind="ExternalOutput")
    zstat_d = nc.dram_tensor("zstat", [P, TILES], F32, kind="ExternalOutput")

    with tile.TileContext(nc) as tc:
        with (
            tc.tile_pool(name="const", bufs=1) as const,
            tc.tile_pool(name="xp", bufs=3) as xp,
            tc.tile_pool(name="lgsb", bufs=2) as lgsb,
            tc.tile_pool(name="outp", bufs=2) as outp,
            tc.tile_pool(name="ep", bufs=4) as ep,
            tc.tile_pool(name="psAB", bufs=2, space="PSUM") as psAB,
            tc.tile_pool(name="psL", bufs=6, space="PSUM") as psL,
        ):
            # identS: rows 0:64 hold I_64, rows 64:128 hold I_64 / 2048.
            identS = const.tile([P, E], F32)
            nc.gpsimd.memset(identS[:], 0.0)
            nc.gpsimd.affine_select(
                out=identS[0:E, :], in_=identS[0:E, :],
                compare_op=ALU.not_equal, fill=1.0,
                base=0, pattern=[[-1, E]], channel_multiplier=1,
            )
            nc.gpsimd.affine_select(
                out=identS[E:2 * E, :], in_=identS[E:2 * E, :],
                compare_op=ALU.not_equal, fill=1.0,
                base=0, pattern=[[-1, E]], channel_multiplier=1,
            )

            whl_sb = const.tile([P, CHUNKS, 2 * E], F16)
            nc.sync.dma_start(whl_sb[:], whl_d.rearrange("(c p) e -> p c e", p=P))
            whz_sb = const.tile([P, CHUNKS, 2 * E], F16)
            nc.sync.dma_start(whz_sb[:], whz_d.rearrange("(c p) e -> p c e", p=P))

            zstat = const.tile([P, TILES], F32)

            for g in range(GROUPS):
                ts_ = slice(g * GROUP, (g + 1) * GROUP)
                # xh on the SP HWDGE ring, xl on the ACT ring: the two input
                # streams interleave across the shared SDMA engines and the
                # per-ring issue gaps stop serializing the stream.
                xh_g = xp.tile([P, CHUNKS, GROUP], F16, name="xh_g")
                nc.sync.dma_start(
                    xh_g[:], xh_d[:, ts_].rearrange("(c p) t -> p c t", p=P)
                )
                xl_g = xp.tile([P, CHUNKS, GROUP], F16, name="xl_g")
                nc.sync.dma_start(
                    xl_g[:], xl_d[:, ts_].rearrange("(c p) t -> p c t", p=P)
                )

                # All matmuls are uniform full-width [128,128] stationary so
                # LDW/MM chains stay pipelined on the PE.
                # whz = [0 | wh]: the zero half adds 0 into rows 0:64.
                AB = psAB.tile([P, GROUP], F32)
                for c in range(CHUNKS):
                    nc.tensor.matmul(
                        AB[:], whl_sb[:, c, :], xh_g[:, c, :],
                        start=(c == 0), stop=False,
                    )
                for c in range(CHUNKS):
                    nc.tensor.matmul(
                        AB[:], whz_sb[:, c, :], xl_g[:, c, :],
                        start=False, stop=(c == CHUNKS - 1),
                    )

                AB_sb = lgsb.tile([P, GROUP], F32)
                nc.vector.tensor_copy(AB_sb[0:E, :], AB[0:E, :])
                nc.vector.tensor_scalar(
                    AB_sb[E:2 * E, :], AB[E:2 * E, :], 1.0 / LO_SCALE, None,
                    op0=ALU.mult,
                )

                gdt = outp.tile([P, TILES_PER_GROUP, E], F32, name="gdt")
                gct = outp.tile([P, TILES_PER_GROUP, E], F32, name="gct")

                for j in range(TILES_PER_GROUP):
                    t_idx = g * TILES_PER_GROUP + j
                    jsl = slice(j * P, (j + 1) * P)
                    lg_ps = psL.tile([P, E], F32)
                    nc.tensor.matmul(
                        lg_ps[:], AB_sb[0:E, jsl], identS[0:E, :],
                        is_transpose=True, start=True, stop=False,
                    )
                    nc.tensor.matmul(
                        lg_ps[:], AB_sb[E:2 * E, jsl], identS[E:2 * E, :],
                        is_transpose=True, start=False, stop=True,
                    )
                    mx = ep.tile([P, 8], F32)
                    nc.vector.max(mx[:], lg_ps[:])
                    negm = ep.tile([P, 1], F32)
                    nc.vector.tensor_scalar_mul(negm[:], mx[:, 0:1], -1.0)

                    et = ep.tile([P, E], F32)
                    ssum = ep.tile([P, 1], F32)
                    nc.scalar.activation(
                        et[:], lg_ps[:], AF.Exp, bias=negm[:], scale=1.0,
                        accum_out=ssum[:],
                    )
                    rec = ep.tile([P, 1], F32)
                    nc.vector.reciprocal(rec[:], ssum[:])
                    probs = ep.tile([P, E], F32)
                    nc.vector.tensor_scalar_mul(probs[:], et[:], rec[:])

                    dt = gdt[:, j, :]
                    nc.vector.tensor_scalar(
                        dt, lg_ps[:], mx[:, 1:2], None, op0=ALU.is_ge
                    )
                    ct = gct[:, j, :]
                    nc.vector.tensor_tensor(ct, probs[:], dt, op=ALU.mult)

                    # z-loss partial: zstat[:, t] = sum_e exp(probs)
                    ee = ep.tile([P, E], F32)
                    nc.scalar.activation(
                        ee[:], probs[:], AF.Exp,
                        accum_out=zstat[:, t_idx:t_idx + 1],
                    )

                nc.scalar.dma_start(
                    disp_d[ts_, :].rearrange("(j p) e -> p j e", p=P), gdt[:]
                )
                nc.scalar.dma_start(
                    comb_d[ts_, :].rearrange("(j p) e -> p j e", p=P), gct[:]
                )

            nc.scalar.dma_start(zstat_d[:], zstat[:])

    nc.compile()
    return nc


def _get_module():
    global _nc
    if _nc is None:
        _nc = _build_module()
    return _nc


def _split16(a):
    hi = a.astype(np.float16)
    lo = ((a - hi.astype(np.float32)) * np.float32(LO_SCALE)).astype(np.float16)
    return hi, lo


def _make_in_maps(hidden_states, gate_weight):
    x = np.asarray(hidden_states, dtype=np.float32).reshape(N, D)
    w = np.asarray(gate_weight, dtype=np.float32)
    wt = np.ascontiguousarray(w.T)  # [D, E]
    wh, wl = _split16(wt)
    whl = np.concatenate([wh, wl], axis=1)               # [D, 128]
    whz = np.concatenate([np.zeros_like(wh), wh], axis=1)  # [D, 128]
    in_maps = []
    for i in range(N_CORES):
        xT = np.ascontiguousarray(x[i * NT:(i + 1) * NT].T)  # [D, NT]
        xh, xl = _split16(xT)
        in_maps.append({"xh": xh, "xl": xl, "whl": whl, "whz": whz})
    return in_maps


def _postprocess(results):
    disp0 = np.concatenate([results[i]["disp"] for i in range(N_CORES)], axis=0)
    comb0 = np.concatenate([results[i]["comb"] for i in range(N_CORES)], axis=0)
    zstat = np.stack([results[i]["zstat"] for i in range(N_CORES)])  # [8,128,16]

    dsum = disp0.sum(axis=0, dtype=np.float64)   # [E]
    csum = comb0.sum(axis=0, dtype=np.float64)   # [E]
    lse = np.log(zstat.astype(np.float64))
    z_loss = np.float32((lse ** 2).mean())

    gates_mean = csum / N
    selection_mean = dsum / N
    lb_loss = np.float32((gates_mean * selection_mean).sum() * E)

    dispatch = np.zeros((N, E, TOPK), np.float32)
    dispatch[:, :, 0] = disp0
    combine = np.zeros((N, E, TOPK), np.float32)
    combine[:, :, 0] = comb0
    return (
        dispatch.reshape(B, S, E, TOPK),
        combine.reshape(B, S, E, TOPK),
        lb_loss,
        z_loss,
    )


def run_on_device(in_maps, trace=False, **kwargs):
    from concourse.bass_utils import run_bass_kernel_spmd

    nc = _get_module()
    return run_bass_kernel_spmd(
        nc, in_maps, list(range(N_CORES)), trace=trace, **kwargs
    )


def kernel(hidden_states, gate_weight):
    in_maps = _make_in_maps(hidden_states, gate_weight)
    res = run_on_device(in_maps)
    return _postprocess(res.results)


# revision 22
# speedup vs baseline: 1.1644x; 1.0555x over previous
"""MoE gate kernel for Trainium2 (8 NeuronCores, token-parallel).

Host side: tokens are sharded 8 ways; each core's activation shard is
transposed to feature-major [D, NT] and split into an fp16 hi/lo pair
(x = hi + lo/2048, both halves fp16, lo pre-scaled by 2^11 to stay in
fp16 normal range) so the PE runs full-rate 16-bit matmuls with ~fp32
logit fidelity.  The gate weight ships as one [D, 128] fp16 block
whose columns are [w_hi | w_lo*2^11], replicated to every core.

Device side (per core), per 512-token group:
  MM1 (full 128-wide stationary [wh|wl]):  PSUM rows 0:64  += wh @ xh
                                           PSUM rows 64:128 += wl @ xh
  MM2 (col-tiled at (0,64), stationary wh): PSUM rows 64:128 += wh @ xl
  De-transpose folds the 2^-11 scale: per 128-token tile the two PSUM
  halves are transposed back through PE with identities [I; I/2048]
  accumulating into logits [128 t, 64 e].  Epilogue (DVE/ACT): max8 ->
  softmax (Exp + fused accumulate) -> top-2 mask (logits >= 2nd max) ->
  combine = probs * mask; per-token softexp sums ship to the host,
  which finishes the scalar aux losses and the per-expert column sums.
"""

import sys

sys.path.insert(0, "/opt/trn_rl_repo")

import numpy as np

B, S, D, E, TOPK = 4, 4096, 2048, 64, 2
N_CORES = 8
N = B * S                 # 16384 tokens
NT = N // N_CORES         # 2048 tokens per core
P = 128                   # partitions
CHUNKS = D // P           # 16 contraction chunks
GROUP = 512               # tokens per matmul group (PSUM free dim)
GROUPS = NT // GROUP      # 4
TILES_PER_GROUP = GROUP // P  # 4
TILES = NT // P           # 16
LO_SCALE = 2048.0         # 2**11: keeps the lo half in fp16 normal range

_nc = None


def _build_module():
    import concourse.bacc as bacc
    import concourse.mybir as mybir
    import concourse.tile as tile

    F32 = mybir.dt.float32
    F16 = mybir.dt.float16
    AF = mybir.ActivationFunctionType
    ALU = mybir.AluOpType

    nc = bacc.Bacc(None, target_bir_lowering=False, debug=False)
    xh_d = nc.dram_tensor("xh", [D, NT], F16, kind="ExternalInput")
    xl_d = nc.dram_tensor("xl", [D, NT], F16, kind="ExternalInput")
    whl_d = nc.dram_tensor("whl", [D, 2 * E], F16, kind="ExternalInput")
    whz_d = nc.dram_tensor("whz", [D, 2 * E], F16, kind="ExternalInput")
    disp_d = nc.dram_tensor("disp", [NT, E], F32, kind="ExternalOutput")
    comb_d = nc.dram_tensor("comb", [NT, E], F32, kind="ExternalOutput")
    zstat_d = nc.dram_tensor("zstat", [P, TILES], F32, kind="ExternalOutput")

    with tile.TileContext(nc) as tc:
        with (
            tc.tile_pool(name="const", bufs=1) as const,
            tc.tile_pool(name="xp", bufs=3) as xp,
            tc.tile_pool(name="lgsb", bufs=2) as lgsb,
            tc.tile_pool(name="outp", bufs=2) as outp,
            tc.tile_pool(name="ep", bufs=4) as ep,
            tc.tile_pool(name="psAB", bufs=2, space="PSUM") as psAB,
            tc.tile_pool(name="psL", bufs=6, space="PSUM") as psL,
        ):
            # identS: rows 0:64 hold I_64, rows 64:128 hold I_64 / 2048.
            identS = const.tile([P, E], F32)
            nc.gpsimd.memset(identS[:], 0.0)
            nc.gpsimd.affine_select(
                out=identS[0:E, :], in_=identS[0:E, :],
                compare_op=ALU.not_equal, fill=1.0,
                base=0, pattern=[[-1, E]], channel_multiplier=1,
            )
            nc.gpsimd.affine_select(
                out=identS[E:2 * E, :], in_=identS[E:2 * E, :],
                compare_op=ALU.not_equal, fill=1.0,
                base=0, pattern=[[-1, E]], channel_multiplier=1,
            )

            whl_sb = const.tile([P, CHUNKS, 2 * E], F16)
            nc.sync.dma_start(whl_sb[:], whl_d.rearrange("(c p) e -> p c e", p=P))
            whz_sb = const.tile([P, CHUNKS, 2 * E], F16)
            nc.sync.dma_start(whz_sb[:], whz_d.rearrange("(c p) e -> p c e", p=P))

            zstat = const.tile([P, TILES], F32)

            # Uneven groups: the final 128-token group minimizes the
            # compute tail left after the input DMA stream finishes.
            GROUP_TILES = [4, 4, 4, 3, 1]
            g0 = 0
            for gt in GROUP_TILES:
                grp = gt * P
                ts_ = slice(g0 * P, g0 * P + grp)
                xh_g = xp.tile([P, CHUNKS, grp], F16, name="xh_g")
                nc.sync.dma_start(
                    xh_g[:], xh_d[:, ts_].rearrange("(c p) t -> p c t", p=P)
                )
                xl_g = xp.tile([P, CHUNKS, grp], F16, name="xl_g")
                nc.sync.dma_start(
                    xl_g[:], xl_d[:, ts_].rearrange("(c p) t -> p c t", p=P)
                )

                # All matmuls are uniform full-width [128,128] stationary so
                # LDW/MM chains stay pipelined on the PE.
                # whz = [0 | wh]: the zero half adds 0 into rows 0:64.
                AB = psAB.tile([P, grp], F32, name="AB")
                for c in range(CHUNKS):
                    nc.tensor.matmul(
                        AB[:], whl_sb[:, c, :], xh_g[:, c, :],
                        start=(c == 0), stop=False,
                    )
                for c in range(CHUNKS):
                    nc.tensor.matmul(
                        AB[:], whz_sb[:, c, :], xl_g[:, c, :],
                        start=False, stop=(c == CHUNKS - 1),
                    )

                AB_sb = lgsb.tile([P, grp], F32, name="AB_sb")
                nc.vector.tensor_copy(AB_sb[0:E, :], AB[0:E, :])
                nc.vector.tensor_scalar(
                    AB_sb[E:2 * E, :], AB[E:2 * E, :], 1.0 / LO_SCALE, None,
                    op0=ALU.mult,
                )

                gdt = outp.tile([P, gt, E], F32, name="gdt")
                gct = outp.tile([P, gt, E], F32, name="gct")

                for j in range(gt):
                    t_idx = g0 + j
                    jsl = slice(j * P, (j + 1) * P)
                    lg_ps = psL.tile([P, E], F32)
                    nc.tensor.matmul(
                        lg_ps[:], AB_sb[0:E, jsl], identS[0:E, :],
                        is_transpose=True, start=True, stop=False,
                    )
                    nc.tensor.matmul(
                        lg_ps[:], AB_sb[E:2 * E, jsl], identS[E:2 * E, :],
                        is_transpose=True, start=False, stop=True,
                    )
                    mx = ep.tile([P, 8], F32)
                    nc.vector.max(mx[:], lg_ps[:])
                    negm = ep.tile([P, 1], F32)
                    nc.vector.tensor_scalar_mul(negm[:], mx[:, 0:1], -1.0)

                    et = ep.tile([P, E], F32)
                    ssum = ep.tile([P, 1], F32)
                    nc.scalar.activation(
                        et[:], lg_ps[:], AF.Exp, bias=negm[:], scale=1.0,
                        accum_out=ssum[:],
                    )
                    rec = ep.tile([P, 1], F32)
                    nc.vector.reciprocal(rec[:], ssum[:])
                    probs = ep.tile([P, E], F32)
                    nc.vector.tensor_scalar_mul(probs[:], et[:], rec[:])

                    dt = gdt[:, j, :]
                    nc.vector.tensor_scalar(
                        dt, lg_ps[:], mx[:, 1:2], None, op0=ALU.is_ge
                    )
                    ct = gct[:, j, :]
                    nc.vector.tensor_tensor(ct, probs[:], dt, op=ALU.mult)

                    # z-loss partial: zstat[:, t] = sum_e exp(probs)
                    ee = ep.tile([P, E], F32)
                    nc.scalar.activation(
                        ee[:], probs[:], AF.Exp,
                        accum_out=zstat[:, t_idx:t_idx + 1],
                    )

                nc.scalar.dma_start(
                    disp_d[ts_, :].rearrange("(j p) e -> p j e", p=P), gdt[:]
                )
                nc.scalar.dma_start(
                    comb_d[ts_, :].rearrange("(j p) e -> p j e", p=P), gct[:]
                )
                g0 += gt

            nc.scalar.dma_start(zstat_d[:], zstat[:])

    nc.compile()
    return nc


def _get_module():
    global _nc
    if _nc is None:
        _nc = _build_module()
    return _nc


def _split16(a):
    hi = a.astype(np.float16)
    lo = ((a - hi.astype(np.float32)) * np.float32(LO_SCALE)).astype(np.float16)
    return hi, lo


def _make_in_maps(hidden_states, gate_weight):
    x = np.asarray(hidden_states, dtype=np.float32).reshape(N, D)
    w = np.asarray(gate_weight, dtype=np.float32)
    wt = np.ascontiguousarray(w.T)  # [D, E]
    wh, wl = _split16(wt)
    whl = np.concatenate([wh, wl], axis=1)               # [D, 128]
    whz = np.concatenate([np.zeros_like(wh), wh], axis=1)  # [D, 128]
    in_maps = []
    for i in range(N_CORES):
        xT = np.ascontiguousarray(x[i * NT:(i + 1) * NT].T)  # [D, NT]
        xh, xl = _split16(xT)
        in_maps.append({"xh": xh, "xl": xl, "whl": whl, "whz": whz})
    return in_maps


def _postprocess(results):
    disp0 = np.concatenate([results[i]["disp"] for i in range(N_CORES)], axis=0)
    comb0 = np.concatenate([results[i]["comb"] for i in range(N_CORES)], axis=0)
    zstat = np.stack([results[i]["zstat"] for i in range(N_CORES)])  # [8,128,16]

    dsum = disp0.sum(axis=0, dtype=np.float64)   # [E]
    csum = comb0.sum(axis=0, dtype=np.float64)   # [E]
    lse = np.log(zstat.astype(np.float64))
    z_loss = np.float32((lse ** 2).mean())

    gates_mean = csum / N
    selection_mean = dsum / N
    lb_loss = np.float32((gates_mean * selection_mean).sum() * E)

    dispatch = np.zeros((N, E, TOPK), np.float32)
    dispatch[:, :, 0] = disp0
    combine = np.zeros((N, E, TOPK), np.float32)
    combine[:, :, 0] = comb0
    return (
        dispatch.reshape(B, S, E, TOPK),
        combine.reshape(B, S, E, TOPK),
        lb_loss,
        z_loss,
    )


def run_on_device(in_maps, trace=False, **kwargs):
    from concourse.bass_utils import run_bass_kernel_spmd

    nc = _get_module()
    return run_bass_kernel_spmd(
        nc, in_maps, list(range(N_CORES)), trace=trace, **kwargs
    )


def kernel(hidden_states, gate_weight):
    in_maps = _make_in_maps(hidden_states, gate_weight)
    res = run_on_device(in_maps)
    return _postprocess(res.results)
